# revision 1
# baseline (speedup 1.0000x reference)
"""Grouped projected head on 8 TRN2 NeuronCores.

Sharding: group axis G=16 split across 8 cores (2 groups/core, expert-parallel).
z is replicated (pre-transposed + bf16-cast on host). Each core computes, for
its two groups g:
    h = z @ W1[g] + b1[g]        -> LayerNorm -> GELU(exact)
    q = h @ W2[g] + b2[g]        -> L2 normalize -> * min(exp(ls[g]),100)
    logits = q @ normalize(Wv[g]).T + bv[g]
and writes its [4096, 8192] fp32 slice of the output; host concatenates.

Per-core dataflow (all matmuls bf16 with fp32 PSUM accumulation):
  mm1:  lhsT = zT block [128i,128b] (stationary), rhs = W1 [128i,512h] -> h psum
        DVE adds b1 + casts to bf16 SBUF; bn_stats/bn_aggr for LN stats;
        (h-mu)*rstd applied in-place via tensor_scalar.
  PE-transpose h tiles -> ACT Gelu with per-partition ln_g/ln_b (affine fused).
  mm2:  lhsT = hT blocks, rhs = W2 -> q psum; DVE adds b2, sumsq, rsqrt via
        ACT Abs_reciprocal_sqrt, scale by min(exp(logit_scale),100) folded in.
  PE-transpose q -> mm3: lhsT = qT blocks, rhs = normalized WvT -> logits psum;
        DVE adds bv and writes fp32 to SBUF; DMA to DRAM.
"""

import sys

sys.path.insert(0, "/opt/trn_rl_repo")

import numpy as np
import ml_dtypes

import concourse.bass as bass
from concourse import bacc, mybir, tile

BF16 = mybir.dt.bfloat16
F32 = mybir.dt.float32
AF = mybir.ActivationFunctionType
ALU = mybir.AluOpType

B, G, IN, HID, PROJ, CHUNK = 4096, 16, 1024, 2048, 256, 4096
NCORES = 8
GPC = G // NCORES          # groups per core
NB = B // 128              # 32 batch blocks
KI = IN // 128             # 8 k-chunks for mm1
TH = HID // 128            # 16 hd-chunks
NVB = CHUNK // 128         # 32 Wv row blocks
NVT = CHUNK // 512         # 8 logits col tiles
LN_EPS = 1e-5
GELU_FUNC = None  # set to AF.Tanh for sim debug

_RT = None  # cached (nc, runner)


def _bc(ap, parts=128):
    """Partition-broadcast a DRAM AP (stride-0 partition dim) for DMA."""
    return bass.AP(tensor=ap.tensor, offset=ap.offset, ap=[[0, parts], *ap.ap])


def _build():
    nc = bacc.Bacc("TRN2", target_bir_lowering=False, debug=False)

    zt_h = nc.dram_tensor("zt", [NB, 128, KI, 128], BF16, kind="ExternalInput")
    w1_h = nc.dram_tensor("w1", [GPC, 128, KI, HID], BF16, kind="ExternalInput")
    w2_h = nc.dram_tensor("w2", [GPC, 128, TH, PROJ], BF16, kind="ExternalInput")
    wv_h = nc.dram_tensor("wv", [GPC, CHUNK, PROJ], BF16, kind="ExternalInput")
    b1_h = nc.dram_tensor("b1", [GPC, HID], BF16, kind="ExternalInput")
    b2_h = nc.dram_tensor("b2", [GPC, PROJ], F32, kind="ExternalInput")
    bv_h = nc.dram_tensor("bv", [GPC, CHUNK], BF16, kind="ExternalInput")
    lng_h = nc.dram_tensor("lng", [128, GPC, TH], F32, kind="ExternalInput")
    lnb_h = nc.dram_tensor("lnb", [128, GPC, TH], F32, kind="ExternalInput")
    ls_h = nc.dram_tensor("ls", [GPC], F32, kind="ExternalInput")
    out_h = nc.dram_tensor("out", [B, GPC * CHUNK], F32, kind="ExternalOutput")

    with tile.TileContext(nc) as tc:
        with (
            tc.tile_pool(name="consts", bufs=1) as consts,
            tc.tile_pool(name="w1p", bufs=2) as w1p,
            tc.tile_pool(name="wtp", bufs=2) as wtp,
            tc.tile_pool(name="ztp", bufs=4) as ztp,
            tc.tile_pool(name="hp", bufs=2) as hp,
            tc.tile_pool(name="htp", bufs=2) as htp,
            tc.tile_pool(name="small", bufs=3) as small,
            tc.tile_pool(name="wvs", bufs=3) as wvs,
            tc.tile_pool(name="lop", bufs=3) as lop,
            tc.tile_pool(name="psA", bufs=2, space="PSUM") as psA,
            tc.tile_pool(name="psT", bufs=3, space="PSUM") as psT,
            tc.tile_pool(name="psQ", bufs=1, space="PSUM") as psQ,
            tc.tile_pool(name="psL", bufs=2, space="PSUM") as psL,
        ):
            # ---------------- prologue: constants ----------------
            from concourse.masks import make_identity

            ident = consts.tile([128, 128], BF16)
            make_identity(nc, ident[:])
            eps_sb = consts.tile([128, 1], F32)
            nc.vector.memset(eps_sb[:], LN_EPS)
            qeps_sb = consts.tile([128, 1], F32)
            nc.vector.memset(qeps_sb[:], 1e-24)

            b1_rep = consts.tile([128, GPC, HID], BF16)
            nc.sync.dma_start(out=b1_rep[:], in_=_bc(b1_h.ap()))
            b2_rep = consts.tile([128, GPC, PROJ], F32)
            nc.sync.dma_start(out=b2_rep[:], in_=_bc(b2_h.ap()))
            bv_rep = consts.tile([128, GPC, CHUNK], BF16)
            nc.sync.dma_start(out=bv_rep[:], in_=_bc(bv_h.ap()))
            lng_sb = consts.tile([128, GPC, TH], F32)
            nc.sync.dma_start(out=lng_sb[:], in_=lng_h.ap())
            lnb_sb = consts.tile([128, GPC, TH], F32)
            nc.sync.dma_start(out=lnb_sb[:], in_=lnb_h.ap())
            w2_sb = consts.tile([128, GPC, TH, PROJ], BF16)
            nc.sync.dma_start(out=w2_sb[:], in_=w2_h.ap().rearrange("g p t n -> p g t n"))
            s_sb = consts.tile([128, GPC], F32)
            nc.sync.dma_start(out=s_sb[:], in_=_bc(ls_h.ap()))
            # s = min(exp(logit_scale), 100)
            nc.scalar.activation(out=s_sb[:], in_=s_sb[:], func=AF.Exp)
            nc.vector.tensor_scalar_min(out=s_sb[:], in0=s_sb[:], scalar1=100.0)

            for gl in range(GPC):
                # ---------------- W1 load ----------------
                w1_sb = w1p.tile([128, KI, HID], BF16)
                nc.sync.dma_start(out=w1_sb[:], in_=w1_h.ap()[gl])

                # ---------------- Wv: row-normalize + transpose ----------------
                wT = wtp.tile([128, 2, CHUNK], BF16)
                for vb in range(NVB):
                    wv_t = wvs.tile([128, PROJ], BF16)
                    nc.sync.dma_start(
                        out=wv_t[:], in_=wv_h.ap()[gl, vb * 128 : (vb + 1) * 128, :]
                    )
                    wjunk = wvs.tile([128, PROJ], BF16)
                    wss = wvs.tile([128, 1], F32)
                    nc.scalar.activation(
                        out=wjunk[:], in_=wv_t[:], func=AF.Square, accum_out=wss[:],
                    )
                    wr = wvs.tile([128, 1], F32)
                    nc.scalar.activation(
                        out=wr[:], in_=wss[:], func=AF.Sqrt, bias=qeps_sb[:],
                    )
                    nc.vector.reciprocal(out=wr[:], in_=wr[:])
                    wn = wvs.tile([128, PROJ], BF16)
                    nc.vector.tensor_scalar_mul(out=wn[:], in0=wv_t[:], scalar1=wr[:])
                    for j in range(2):
                        ptw = psT.tile([128, 128], BF16, tag="pt")
                        nc.tensor.transpose(
                            out=ptw[:], in_=wn[:, j * 128 : (j + 1) * 128],
                            identity=ident[:],
                        )
                        nc.vector.tensor_copy(
                            out=wT[:, j, vb * 128 : (vb + 1) * 128], in_=ptw[:]
                        )

                # ---------------- main loop over batch blocks ----------------
                for bb in range(NB):
                    zt_t = ztp.tile([128, KI, 128], BF16)
                    nc.sync.dma_start(out=zt_t[:], in_=zt_h.ap()[bb])

                    # mm1: h = z @ W1 (+b1), into 4 psum tiles of [128, 512]
                    h_sb = hp.tile([128, HID], BF16)
                    stats = small.tile([128, 4, 6], F32)
                    for nt in range(4):
                        ph = psA.tile([128, 512], F32)
                        for k in range(KI):
                            nc.tensor.matmul(
                                ph[:], zt_t[:, k, :],
                                w1_sb[:, k, nt * 512 : (nt + 1) * 512],
                                start=(k == 0), stop=(k == KI - 1),
                            )
                        hs = h_sb[:, nt * 512 : (nt + 1) * 512]
                        nc.vector.tensor_tensor(
                            out=hs, in0=ph[:],
                            in1=b1_rep[:, gl, nt * 512 : (nt + 1) * 512], op=ALU.add,
                        )
                        nc.vector.bn_stats(out=stats[:, nt, :], in_=hs)

                    mv = small.tile([128, 2], F32)
                    nc.vector.bn_aggr(out=mv[:], in_=stats[:])
                    rstd = small.tile([128, 1], F32)
                    nc.scalar.activation(
                        out=rstd[:], in_=mv[:, 1:2], func=AF.Sqrt, bias=eps_sb[:],
                    )
                    nc.vector.reciprocal(out=rstd[:], in_=rstd[:])
                    # h = (h - mu) * rstd, in place
                    nc.vector.tensor_scalar(
                        out=h_sb[:], in0=h_sb[:], scalar1=mv[:, 0:1], scalar2=rstd[:],
                        op0=ALU.subtract, op1=ALU.mult,
                    )

                    # transpose + fused LN-affine + exact GELU
                    hT = htp.tile([128, TH, 128], BF16)
                    for t in range(TH):
                        pt = psT.tile([128, 128], BF16, tag="pt")
                        nc.tensor.transpose(
                            out=pt[:], in_=h_sb[:, t * 128 : (t + 1) * 128],
                            identity=ident[:],
                        )
                        nc.scalar.activation(
                            out=hT[:, t, :], in_=pt[:], func=(GELU_FUNC or AF.Gelu),
                            scale=lng_sb[:, gl, t : t + 1],
                            bias=lnb_sb[:, gl, t : t + 1],
                        )

                    # mm2: q = h @ W2
                    pq = psQ.tile([128, PROJ], F32)
                    for t in range(TH):
                        nc.tensor.matmul(
                            pq[:], hT[:, t, :], w2_sb[:, gl, t, :],
                            start=(t == 0), stop=(t == TH - 1),
                        )
                    q_sb = small.tile([128, PROJ], F32)
                    nc.vector.tensor_tensor(
                        out=q_sb[:], in0=pq[:], in1=b2_rep[:, gl, :], op=ALU.add
                    )
                    qjunk = small.tile([128, PROJ], F32)
                    qss = small.tile([128, 1], F32)
                    nc.scalar.activation(
                        out=qjunk[:], in_=q_sb[:], func=AF.Square, accum_out=qss[:],
                    )
                    rq = small.tile([128, 1], F32)
                    nc.scalar.activation(
                        out=rq[:], in_=qss[:], func=AF.Sqrt, bias=qeps_sb[:],
                    )
                    nc.vector.reciprocal(out=rq[:], in_=rq[:])
                    qsc = small.tile([128, 1], F32)
                    nc.vector.tensor_tensor(
                        out=qsc[:], in0=rq[:], in1=s_sb[:, gl : gl + 1], op=ALU.mult
                    )
                    qn = small.tile([128, PROJ], BF16)
                    nc.vector.tensor_scalar_mul(out=qn[:], in0=q_sb[:], scalar1=qsc[:])
                    qT = small.tile([128, 2, 128], BF16)
                    for j in range(2):
                        ptq = psT.tile([128, 128], BF16, tag="pt")
                        nc.tensor.transpose(
                            out=ptq[:], in_=qn[:, j * 128 : (j + 1) * 128],
                            identity=ident[:],
                        )
                        nc.vector.tensor_copy(out=qT[:, j, :], in_=ptq[:])

                    # mm3: logits = q @ wT (+bv), 8 tiles of 512, fp32 out
                    for vh in range(2):
                        lo = lop.tile([128, 4, 512], F32)
                        for v4 in range(4):
                            vt = vh * 4 + v4
                            pl = psL.tile([128, 512], F32)
                            nc.tensor.matmul(
                                pl[:], qT[:, 0, :],
                                wT[:, 0, vt * 512 : (vt + 1) * 512],
                                start=True, stop=False,
                            )
                            nc.tensor.matmul(
                                pl[:], qT[:, 1, :],
                                wT[:, 1, vt * 512 : (vt + 1) * 512],
                                start=False, stop=True,
                            )
                            nc.vector.tensor_tensor(
                                out=lo[:, v4, :], in0=pl[:],
                                in1=bv_rep[:, gl, vt * 512 : (vt + 1) * 512],
                                op=ALU.add,
                            )
                        nc.sync.dma_start(
                            out=out_h.ap()[
                                bb * 128 : (bb + 1) * 128,
                                gl * CHUNK + vh * 2048 : gl * CHUNK + (vh + 1) * 2048,
                            ],
                            in_=lo[:].rearrange("p a b -> p (a b)"),
                        )

    nc.compile()
    return nc


def _make_runner(nc):
    """Reusable jitted SPMD executor (mirrors bass2jax.run_bass_via_pjrt)."""
    import jax
    from jax.sharding import Mesh, PartitionSpec, NamedSharding
    from jax.experimental.shard_map import shard_map
    from concourse.bass2jax import _bass_exec_p, partition_id_tensor, install_neuronx_cc_hook

    install_neuronx_cc_hook()
    partition_name = nc.partition_id_tensor.name if nc.partition_id_tensor else None
    in_names, out_names, out_avals = [], [], []
    for alloc in nc.m.functions[0].allocations:
        if not isinstance(alloc, mybir.MemoryLocationSet):
            continue
        name = alloc.memorylocations[0].name
        if alloc.kind == "ExternalInput":
            if name != partition_name:
                in_names.append(name)
        elif alloc.kind == "ExternalOutput":
            out_names.append(name)
            out_avals.append(
                jax.core.ShapedArray(tuple(alloc.tensor_shape), mybir.dt.np(alloc.dtype))
            )
    n_params = len(in_names)
    all_in_names = in_names + out_names
    if partition_name is not None:
        all_in_names.append(partition_name)

    def _body(*args):
        operands = list(args)
        if partition_name is not None:
            operands.append(partition_id_tensor())
        return tuple(
            _bass_exec_p.bind(
                *operands,
                out_avals=tuple(out_avals),
                in_names=tuple(all_in_names),
                out_names=tuple(out_names),
                lowering_input_output_aliases=(),
                sim_require_finite=True,
                sim_require_nnan=True,
                nc=nc,
            )
        )

    devices = jax.devices()[:NCORES]
    mesh = Mesh(np.asarray(devices), ("core",))
    spec = NamedSharding(mesh, PartitionSpec("core"))
    n_out = len(out_names)
    fn = jax.jit(
        shard_map(
            _body, mesh=mesh,
            in_specs=(PartitionSpec("core"),) * (n_params + n_out),
            out_specs=(PartitionSpec("core"),) * n_out,
            check_rep=False,
        ),
        keep_unused=True,
    )

    def put(in_maps):
        import jax as _jax
        concat = [
            _jax.device_put(
                np.concatenate([np.asarray(in_maps[c][nm]) for c in range(NCORES)], axis=0),
                spec,
            )
            for nm in in_names
        ]
        zeros = [
            _jax.device_put(
                np.zeros((NCORES * a.shape[0], *a.shape[1:]), a.dtype), spec
            )
            for a in out_avals
        ]
        return concat + zeros

    def run(args):
        outs = fn(*args)
        return outs, out_names, out_avals

    return put, run


def _prep_inputs(z, W1, b1, ln_g, ln_b, W2, b2, Wv, bv, logit_scale):
    bf = ml_dtypes.bfloat16
    zt = np.ascontiguousarray(
        z.T.reshape(KI, 128, NB, 128).transpose(2, 1, 0, 3)
    ).astype(bf)  # [bb, p, k, b]
    in_maps = []
    for c in range(NCORES):
        gs = slice(GPC * c, GPC * (c + 1))
        w1c = np.ascontiguousarray(
            W1[gs].reshape(GPC, KI, 128, HID).transpose(0, 2, 1, 3)
        ).astype(bf)  # [g, p, k, n]
        w2c = np.ascontiguousarray(
            W2[gs].reshape(GPC, TH, 128, PROJ).transpose(0, 2, 1, 3)
        ).astype(bf)  # [g, p, t, n]
        lngc = np.ascontiguousarray(
            ln_g[gs].reshape(GPC, TH, 128).transpose(2, 0, 1)
        ).astype(np.float32)  # [p, g, t]
        lnbc = np.ascontiguousarray(
            ln_b[gs].reshape(GPC, TH, 128).transpose(2, 0, 1)
        ).astype(np.float32)
        in_maps.append(
            {
                "zt": zt,
                "w1": w1c,
                "w2": w2c,
                "wv": Wv[gs].astype(bf),
                "b1": b1[gs].astype(bf),
                "b2": b2[gs].astype(np.float32),
                "bv": bv[gs].astype(bf),
                "lng": lngc,
                "lnb": lnbc,
                "ls": logit_scale[gs].astype(np.float32),
            }
        )
    return in_maps


def _get_runtime():
    global _RT
    if _RT is None:
        nc = _build()
        put, run = _make_runner(nc)
        _RT = (nc, put, run)
    return _RT


def kernel(**inputs):
    inputs = {k: np.asarray(v) for k, v in inputs.items()}
    in_maps = _prep_inputs(**inputs)
    _, put, run = _get_runtime()
    args = put(in_maps)
    outs, out_names, out_avals = run(args)
    out = np.asarray(outs[out_names.index("out")])
    out = out.reshape(NCORES, B, GPC * CHUNK)
    return np.concatenate(list(out), axis=1).astype(np.float32)

